# revision 55
# baseline (speedup 1.0000x reference)
"""GCN block (2-layer) Trainium2 Bass kernel.

Math (per B*T slice, shared graph):
  t2 = relu(A @ X @ W1 + b1);  out = sigmoid(A @ t2 @ W2 + b2)
  A = D^-1/2 (Adj + I) D^-1/2  (PyG gcn_norm, counts edge multiplicity)

Device mapping:
  Both layers share one structure: a dense A application followed by a
  feature transform. A is applied as dense 128x128 blocks of M = 8*A in
  fp8e4 (x8 keeps every nonzero coefficient in fp8-normal range; the 1/8
  is folded exactly into the activation's scale) via PE matmuls in fp8
  DoubleRow mode (K = 256 src nodes per matmul) accumulating in PSUM.
  The moving operand is X itself (layer 1) or t2 (layer 2), node-major
  fp8 — no separate X@W1 front-end exists. The feature transform runs
  per dst block: PE-transpose 128x128 pair-blocks (via identity matmul),
  then one matmul against blockdiag(W,W) in bf16, then the activation
  applies scale=1/8 and the bias (per-partition, exact f32). Layer 1
  PE-transposes the result back to node-major for storage; layer 2's
  [features, nodes] output is the wanted OUT layout directly.

Sharding: each of 8 cores owns 10 of the 80 dst-node blocks (128 nodes
each, N padded 10000->10240) for ALL 24 B*T slices; the relu'd layer-1
activations are AllGathered between the layers in three 512-column
chain chunks so comm overlaps compute.

Layout: the 1536 free columns (24 slices x 64 features, col = s*64+f)
are processed in 3 chains of 512. M stays SBUF-resident (loaded once);
the moving X/t2 operands live in per-chain sets of 10 tiles
[128, 8 node blocks, 512] fp8, double-buffered (2 sets), so A-stages
start on the first node blocks while later ones still stream in.
Each chain stage is software-pipelined: per step the PE runs
[A-matmuls(bi), W-matmul(bi-2), transposes(bi-1), back-transposes]
so it never waits on the psum<->sbuf copies (DVE/Act).
"""
import time

import numpy as np
import ml_dtypes

import concourse.bacc as bacc
import concourse.mybir as mybir
import concourse.tile as tile
from concourse.bass_utils import run_bass_kernel_spmd

N_CORES = 8
N = 10000
NP = 10240            # padded nodes
NB = NP // 128        # 80 node blocks
NB2 = NB // 2         # 40 src-block pairs (DoubleRow K=256)
NT = 10               # tiles per set
TB = NB // NT         # 8 node blocks per set tile
BPC = NB // N_CORES   # 10 dst blocks per core
B, T, C = 2, 12, 64
S = B * T             # 24 slices
F = S * C             # 1536 free columns
PAIRS = S // 2        # 12 slice pairs
NCH = 3               # chains (column chunks)
CW = F // NCH         # 512 cols per chain

f32 = mybir.dt.float32
bf16 = mybir.dt.bfloat16
fp8 = mybir.dt.float8e4
DR = mybir.MatmulPerfMode.DoubleRow
AF = mybir.ActivationFunctionType


def build_program(with_collective=True, nc_hook=None):
    nc = bacc.Bacc("TRN2", target_bir_lowering=False, debug=False,
                   num_devices=N_CORES)
    if nc_hook is not None:
        nc_hook(nc)

    # X node-major: [sb][p][col], col = s*64+cin, fp8 (raw X, no folds)
    x_ext = nc.dram_tensor("XN", [NB, 128, F], fp8, kind="ExternalInput")
    # M rows: [bi][p_src][(j2,a)*128+dst], fp8 = 8*A (both dinv folded)
    m_ext = nc.dram_tensor("M", [BPC, 128, NB * 128], fp8,
                           kind="ExternalInput")
    w1_ext = nc.dram_tensor("W1D", [128, 128], bf16, kind="ExternalInput")
    w2_ext = nc.dram_tensor("W2D", [128, 128], bf16, kind="ExternalInput")
    b1_ext = nc.dram_tensor("B1", [128, 1], f32, kind="ExternalInput")
    b2_ext = nc.dram_tensor("B2", [128, 1], f32, kind="ExternalInput")
    id_ext = nc.dram_tensor("IDT", [128, 128], bf16, kind="ExternalInput")
    out_ext = nc.dram_tensor("OUT", [PAIRS, 128, BPC * 128], f32,
                             kind="ExternalOutput")

    with tile.TileContext(nc) as tc:
        with (
            tc.tile_pool(name="consts", bufs=1) as consts,
            tc.tile_pool(name="mres", bufs=4 * BPC) as pool_m,
            tc.tile_pool(name="set", bufs=2 * NT) as pool_set,
            tc.tile_pool(name="sA", bufs=2) as pool_sa,
            tc.tile_pool(name="sT", bufs=2) as pool_st,
            tc.tile_pool(name="ro", bufs=2) as pool_ro,
            tc.tile_pool(name="t2t", bufs=4) as pool_t2,
            tc.tile_pool(name="ot", bufs=2) as pool_ot,
            tc.tile_pool(name="pa", bufs=4, space="PSUM") as pool_pa,
            tc.tile_pool(name="pT", bufs=1, space="PSUM") as pool_pt,
            tc.tile_pool(name="pw", bufs=2, space="PSUM") as pool_pw,
            tc.tile_pool(name="pTb", bufs=1, space="PSUM") as pool_ptb,
            tc.tile_pool(name="dram", bufs=1, space="DRAM") as dram,
        ):
            # constants (Pool/SWDGE queue: keeps HWDGE free for X loads)
            w1t = consts.tile([128, 128], bf16, tag="w1")
            nc.gpsimd.dma_start(w1t[:], w1_ext[:])
            w2t = consts.tile([128, 128], bf16, tag="w2")
            nc.gpsimd.dma_start(w2t[:], w2_ext[:])
            b1t = consts.tile([128, 1], f32, tag="b1")
            nc.gpsimd.dma_start(b1t[:], b1_ext[:])
            b2t = consts.tile([128, 1], f32, tag="b2")
            nc.gpsimd.dma_start(b2t[:], b2_ext[:])
            idt = consts.tile([128, 128], bf16, tag="id")
            nc.gpsimd.dma_start(idt[:], id_ext[:])

            # M resident, 4 j2-range tiles per dst block: [128, j2, a, dst].
            # Finer tiles interleave with quad loads on the DMA FIFO and
            # let the A wavefront start after a 0.9us piece, not 3.6us.
            MP = 4
            MJ = NB2 // MP
            mres = [[pool_m.tile([128, MJ, 2, 128], fp8, tag="m",
                                 name=f"m{bi}_{p}") for p in range(MP)]
                    for bi in range(BPC)]
            m_loaded = [[False] * MP for _ in range(BPC)]

            def load_m_part(bi, p, eng=None):
                if not m_loaded[bi][p]:
                    (eng or nc.sync).dma_start(
                        mres[bi][p][:].rearrange("p a b q -> p (a b q)"),
                        m_ext[bi, :, p * MJ * 256:(p + 1) * MJ * 256])
                    m_loaded[bi][p] = True

            def load_m(bi, eng=None):
                for p in range(MP):
                    load_m_part(bi, p, eng)

            # DRAM intermediates (per chain)
            t2loc = [dram.tile([BPC * 128, CW], fp8, tag=f"t2loc{c}",
                               name=f"t2loc{c}")
                     for c in range(NCH)]
            if with_collective:
                t2full = [dram.tile([NP, CW], fp8, tag=f"t2full{c}",
                                    name=f"t2full{c}", addr_space="Shared")
                          for c in range(NCH)]
            else:
                # per-tile pieces: fine-grained comm->load dependencies in
                # the timing surrogate (same total receive traffic)
                t2full = [[dram.tile([TB * 128, CW], fp8, tag=f"t2f{c}_{q}",
                                     name=f"t2f{c}_{q}")
                           for q in range(NT)] for c in range(NCH)]

            def new_set(label):
                return [pool_set.tile([128, TB, CW], fp8, tag="set",
                                      name=f"{label}_q{q}")
                        for q in range(NT)]

            def x_load(c, st, q):
                nc.sync.dma_start(
                    st[q][:],
                    x_ext[q * TB:(q + 1) * TB, :, c * CW:(c + 1) * CW]
                    .rearrange("a p f -> p a f"))

            def t2c_load(c, st, q):
                if with_collective:
                    src = (t2full[c][q * TB * 128:(q + 1) * TB * 128, :]
                           .rearrange("(a p) f -> p a f", p=128))
                else:
                    src = t2full[c][q][:].rearrange("(a p) f -> p a f",
                                                    p=128)
                nc.sync.dma_start(st[q][:], src)

            def comm_write(c, r):
                """r-th piece of the chain-c AllGather, emitted spread out
                so its DMA traffic interleaves with compute-window DMA."""
                if with_collective:
                    if r == 0:
                        nc.gpsimd.collective_compute(
                            "AllGather", mybir.AluOpType.bypass,
                            replica_groups=[list(range(N_CORES))],
                            ins=[t2loc[c][:]], outs=[t2full[c][:]])
                else:
                    # timing stand-in: emulate receive traffic, per piece
                    # (timing-only; contents unchecked in this build)
                    nc.gpsimd.dma_start(t2full[c][r][:],
                                        t2loc[c][0:TB * 128, :])

            NCW = 1 if with_collective else NT  # comm pieces per chain

            def a_mms(st, bi):
                """psum [128, CW] = sum_j2 M[bi,j2] @ st[j2]."""
                ps = pool_pa.tile([128, CW], f32, tag="pa")
                h = TB // 2  # j2 pairs per set tile
                for j2 in range(NB2):
                    nc.tensor.matmul(
                        ps[:], mres[bi][j2 // MJ][:, j2 % MJ],
                        st[j2 // h][:, 2 * (j2 % h):2 * (j2 % h) + 2, :],
                        start=(j2 == 0), stop=(j2 == NB2 - 1),
                        perf_mode=DR)
                return ps

            def chain_stage(c, layer, st, extra=None, fill=False,
                            last=False):
                """One chain of one layer, software-pipelined across bi.
                Per step: PE runs [A(bi), Wmm(bi-2), T(bi-1), Tback(bi-3)];
                DVE/Act run the psum<->sbuf copies a step behind.
                fill=True: the first 4 dst blocks accumulate j2-major so PE
                tracks the arriving X tiles instead of idling (pipeline
                fill at program start)."""
                wt = w1t if layer == 1 else w2t
                bt = b1t if layer == 1 else b2t
                act = AF.Relu if layer == 1 else AF.Sigmoid
                sAs, sTs, ros = {}, {}, {}

                pss = {}
                if fill:
                    NF = 4
                    h = TB // 2
                    for bi in range(NF):
                        pss[bi] = pool_pa.tile([128, CW], f32, tag="pa",
                                               name=f"paf{bi}")
                    # diagonal (tile+bi) order: each accumulator advances as
                    # soon as its X tile and M piece have landed
                    for s in range(NT + NF - 1):
                        for bi in range(NF):
                            q = s - bi
                            if not 0 <= q < NT:
                                continue
                            for j2 in range(q * h, (q + 1) * h):
                                nc.tensor.matmul(
                                    pss[bi][:], mres[bi][j2 // MJ][:, j2 % MJ],
                                    st[q][:, 2 * (j2 % h):2 * (j2 % h) + 2, :],
                                    start=(j2 == 0), stop=(j2 == NB2 - 1),
                                    perf_mode=DR)

                def aux(i, dst, src):
                    if i % 2 == 0:
                        nc.vector.tensor_scalar_mul(dst, src, 1.0)
                    else:
                        nc.scalar.activation(dst, src, AF.Copy)

                def s_t1(bi):  # transposes of sA(bi) + copy to sbuf
                    pT = pool_pt.tile([128, CW], bf16, tag="pT")
                    for pl in range(4):
                        nc.tensor.transpose(
                            pT[:, pl * 128:(pl + 1) * 128],
                            sAs[bi][:, pl * 128:(pl + 1) * 128], idt[:])
                    sT = pool_st.tile([128, CW], bf16, tag="sT")
                    aux(bi + 1, sT[:], pT[:])
                    sTs[bi] = sT

                def s_w(bi):  # W matmul + activation
                    ps = pool_pw.tile([128, CW], f32, tag="pw")
                    nc.tensor.matmul(ps[:], wt[:], sTs[bi][:],
                                     start=True, stop=True)
                    if layer == 1:
                        ro = pool_ro.tile([128, CW], bf16, tag="ro")
                        nc.scalar.activation(ro[:], ps[:], act,
                                             scale=0.125, bias=bt[:])
                        ros[bi] = ro
                    else:
                        ot = pool_ot.tile([128, 4, 128], f32, tag="ot")
                        nc.scalar.activation(
                            ot[:].rearrange("p a n -> p (a n)"), ps[:],
                            act, scale=0.125, bias=bt[:])
                        nc.gpsimd.dma_start(
                            out_ext[4 * c:4 * c + 4, :,
                                    bi * 128:(bi + 1) * 128]
                            .rearrange("a p n -> p a n"), ot[:])

                def s_tb(bi):  # layer 1: transpose back to node-major
                    pb = pool_ptb.tile([128, CW], bf16, tag="pTb")
                    for pl in range(4):
                        nc.tensor.transpose(
                            pb[:, pl * 128:(pl + 1) * 128],
                            ros[bi][:, pl * 128:(pl + 1) * 128], idt[:])
                    t2t = pool_t2.tile([128, CW], fp8, tag="t2t")
                    aux(bi, t2t[:], pb[:])
                    nc.gpsimd.dma_start(
                        t2loc[c][bi * 128:(bi + 1) * 128, :], t2t[:])

                depth = 4 if layer == 1 else 3
                for step in range(BPC + depth):
                    if step < BPC:
                        ps = pss.get(step)
                        if ps is None:
                            ps = a_mms(st, step)
                        sA = pool_sa.tile([128, CW], bf16, tag="sA")
                        aux(step, sA[:], ps[:])
                        sAs[step] = sA
                    if 2 <= step < BPC + 2:
                        s_w(step - 2)
                    if 1 <= step < BPC + 1:
                        s_t1(step - 1)
                    if layer == 1 and 3 <= step < BPC + 3:
                        s_tb(step - 3)
                    if extra is not None:
                        extra(step)

            # ---- pipeline schedule ----
            # fill: chain-0 X tiles stream in (the A wavefront follows
            # them); M0/M1 slot into the stream early, the rest spread
            # one per pipeline step so they never starve the quad loads
            # fill: X tiles stream in with the M parts the j2-major fill
            # blocks (bi 0-3) need, paced piece-wise so neither starves
            st0 = new_set("x0")
            for q in range(NT):
                x_load(0, st0, q)
                # one M piece per X tile: piece k=(p,bi) in bi-major order
                for k in ([q] if q < 8 else [8 + 2 * (q - 8),
                                             9 + 2 * (q - 8)]):
                    if k < 16:
                        load_m_part(k % 4, k // 4, nc.scalar)
            for bi in range(4):
                load_m_part(bi, 3, nc.scalar)

            st1 = new_set("x1")

            def fx1(step):
                # SP queue: 1:1 interleave with next chain's X paces the
                # M streams without blocking any aux-op queue
                if step < NT:
                    x_load(1, st1, step)
                if step + 4 < BPC:
                    load_m(step + 4)
            chain_stage(0, 1, st0, fx1, fill=True)

            def fc0(step):
                if step < NCW:
                    comm_write(0, step)
            chain_stage(1, 1, st1, fc0)

            stl2_0 = new_set("t2c0")
            for q in range(NT):
                t2c_load(0, stl2_0, q)

            st2 = new_set("x2")

            def f20(step):
                if step < NT:
                    x_load(2, st2, step)
                if step < NCW:
                    comm_write(1, step)
            chain_stage(0, 2, stl2_0, f20)

            stl2_1 = new_set("t2c1")

            def f12(step):
                if step < NT:
                    t2c_load(1, stl2_1, step)
            chain_stage(2, 1, st2, f12)

            def f21(step):
                if step < NCW:
                    comm_write(2, step)
            chain_stage(1, 2, stl2_1, f21)

            stl2_2 = new_set("t2c2")
            for q in range(NT):
                t2c_load(2, stl2_2, q)
            chain_stage(2, 2, stl2_2, last=True)

    nc.compile()
    return nc


def prepare_inputs(X, edge_index, W1, b1, W2, b2):
    """Host-side graph/layout prep. Returns per-core in_maps."""
    X = np.asarray(X, dtype=np.float32)
    edge_index = np.asarray(edge_index)
    W1 = np.asarray(W1, dtype=np.float32)
    b1 = np.asarray(b1, dtype=np.float32)
    W2 = np.asarray(W2, dtype=np.float32)
    b2 = np.asarray(b2, dtype=np.float32)

    src = edge_index[0].astype(np.int64)
    dst = edge_index[1].astype(np.int64)

    deg = np.bincount(dst, minlength=N).astype(np.float32) + 1.0
    dinv = 1.0 / np.sqrt(deg)
    dinv_pad = np.zeros(NP, np.float32)
    dinv_pad[:N] = dinv

    # M = 8 * D^-1/2 (Adj + I) D^-1/2; x8 keeps coefficients in fp8e4
    # normal range (compensated by activation scale=1/8)
    Mfull = np.zeros((NP, NP), np.float32)
    np.add.at(Mfull, (dst, src), 1.0)
    Mfull[np.arange(N), np.arange(N)] += 1.0
    Mfull *= 8.0 * dinv_pad[:, None] * dinv_pad[None, :]

    # XN: [sb][p][col], col = s*64+cin with s = b*T+t, raw X in fp8
    XT = np.zeros((NP, F), np.float32)
    XT[:N] = np.transpose(X, (1, 0, 2, 3)).reshape(N, F)
    XN = XT.reshape(NB, 128, F).astype(ml_dtypes.float8_e4m3)

    def blockdiag(W):
        D = np.zeros((128, 128), np.float32)
        D[:64, :64] = W
        D[64:, 64:] = W
        return D.astype(ml_dtypes.bfloat16)

    W1D = blockdiag(W1)
    W2D = blockdiag(W2)
    B1 = np.concatenate([b1, b1])[:, None].astype(np.float32)
    B2 = np.concatenate([b2, b2])[:, None].astype(np.float32)
    IDT = np.eye(128, dtype=ml_dtypes.bfloat16)

    in_maps = []
    for c in range(N_CORES):
        rows = Mfull[c * BPC * 128:(c + 1) * BPC * 128, :]
        Mc = rows.reshape(BPC, 128, NB, 128).transpose(0, 3, 2, 1)
        Mc = np.ascontiguousarray(Mc).reshape(BPC, 128, NB * 128)
        Mc = Mc.astype(ml_dtypes.float8_e4m3)
        in_maps.append({"XN": XN, "M": Mc, "W1D": W1D, "W2D": W2D,
                        "B1": B1, "B2": B2, "IDT": IDT})
    return in_maps


_NC_CACHE = {}


def kernel(X, edge_index, W1, b1, W2, b2):
    if "nc" not in _NC_CACHE:
        _NC_CACHE["nc"] = build_program(with_collective=True)
    nc = _NC_CACHE["nc"]
    in_maps = prepare_inputs(X, edge_index, W1, b1, W2, b2)

    res = None
    for attempt in range(5):
        try:
            res = run_bass_kernel_spmd(nc, in_maps, list(range(N_CORES)))
            break
        except Exception:
            if attempt == 4:
                raise
            time.sleep(60.0 * (attempt + 1))
    assert res is not None

    # reassemble: per core [12, 128, 1280] -> [24, 64, 1280]
    full = np.zeros((S, C, N), np.float32)
    for c in range(N_CORES):
        o = res.results[c]["OUT"].reshape(S, C, BPC * 128)
        lo = c * BPC * 128
        hi = min(N, (c + 1) * BPC * 128)
        if lo < N:
            full[:, :, lo:hi] = o[:, :, :hi - lo]
    out = full.reshape(B, T, C, N).transpose(0, 3, 1, 2)
    return np.ascontiguousarray(out)


# revision 59
# speedup vs baseline: 1.0299x; 1.0299x over previous
"""GCN block (2-layer) Trainium2 Bass kernel.

Math (per B*T slice, shared graph):
  t2 = relu(A @ X @ W1 + b1);  out = sigmoid(A @ t2 @ W2 + b2)
  A = D^-1/2 (Adj + I) D^-1/2  (PyG gcn_norm, counts edge multiplicity)

Device mapping:
  Both layers share one structure: a dense A application followed by a
  feature transform. A is applied as dense 128x128 blocks of M = 8*A in
  fp8e4 (x8 keeps every nonzero coefficient in fp8-normal range; the 1/8
  is folded exactly into the activation's scale) via PE matmuls in fp8
  DoubleRow mode (K = 256 src nodes per matmul) accumulating in PSUM.
  The moving operand is X itself (layer 1) or t2 (layer 2), node-major
  fp8 — no separate X@W1 front-end exists. The feature transform runs
  per dst block: PE-transpose 128x128 pair-blocks (via identity matmul),
  then one matmul against blockdiag(W,W) in bf16, then the activation
  applies scale=1/8 and the bias (per-partition, exact f32). Layer 1
  PE-transposes the result back to node-major for storage; layer 2's
  [features, nodes] output is the wanted OUT layout directly.

Sharding: each of 8 cores owns 10 of the 80 dst-node blocks (128 nodes
each, N padded 10000->10240) for ALL 24 B*T slices; the relu'd layer-1
activations are AllGathered between the layers in three 512-column
chain chunks so comm overlaps compute.

Layout: the 1536 free columns (24 slices x 64 features, col = s*64+f)
are processed in 3 chains of 512. M stays SBUF-resident (loaded once);
the moving X/t2 operands live in per-chain sets of 10 tiles
[128, 8 node blocks, 512] fp8, double-buffered (2 sets), so A-stages
start on the first node blocks while later ones still stream in.
Each chain stage is software-pipelined: per step the PE runs
[A-matmuls(bi), W-matmul(bi-2), transposes(bi-1), back-transposes]
so it never waits on the psum<->sbuf copies (DVE/Act).
"""
import time

import numpy as np
import ml_dtypes

import concourse.bacc as bacc
import concourse.mybir as mybir
import concourse.tile as tile
from concourse.bass_utils import run_bass_kernel_spmd

N_CORES = 8
N = 10000
NP = 10240            # padded nodes
NB = NP // 128        # 80 node blocks
NB2 = NB // 2         # 40 src-block pairs (DoubleRow K=256)
NT = 10               # tiles per set
TB = NB // NT         # 8 node blocks per set tile
BPC = NB // N_CORES   # 10 dst blocks per core
B, T, C = 2, 12, 64
S = B * T             # 24 slices
F = S * C             # 1536 free columns
PAIRS = S // 2        # 12 slice pairs
NCH = 3               # chains (column chunks)
CW = F // NCH         # 512 cols per chain

f32 = mybir.dt.float32
bf16 = mybir.dt.bfloat16
fp8 = mybir.dt.float8e4
DR = mybir.MatmulPerfMode.DoubleRow
AF = mybir.ActivationFunctionType


def build_program(with_collective=True, nc_hook=None):
    nc = bacc.Bacc("TRN2", target_bir_lowering=False, debug=False,
                   num_devices=N_CORES)
    if nc_hook is not None:
        nc_hook(nc)

    # X node-major: [sb][p][col], col = s*64+cin, fp8 (raw X, no folds)
    x_ext = nc.dram_tensor("XN", [NB, 128, F], fp8, kind="ExternalInput")
    # M rows: [bi][p_src][(j2,a)*128+dst], fp8 = 8*A (both dinv folded)
    m_ext = nc.dram_tensor("M", [BPC, 128, NB * 128], fp8,
                           kind="ExternalInput")
    w1_ext = nc.dram_tensor("W1D", [128, 128], bf16, kind="ExternalInput")
    w2_ext = nc.dram_tensor("W2D", [128, 128], bf16, kind="ExternalInput")
    b1_ext = nc.dram_tensor("B1", [128, 1], f32, kind="ExternalInput")
    b2_ext = nc.dram_tensor("B2R", [128, F], f32, kind="ExternalInput")
    id_ext = nc.dram_tensor("IDT", [128, 128], bf16, kind="ExternalInput")
    out_ext = nc.dram_tensor("OUT", [BPC * 128, F], f32,
                             kind="ExternalOutput")

    with tile.TileContext(nc) as tc:
        with (
            tc.tile_pool(name="consts", bufs=1) as consts,
            tc.tile_pool(name="mres", bufs=4 * BPC) as pool_m,
            tc.tile_pool(name="set", bufs=2 * NT) as pool_set,
            tc.tile_pool(name="sA", bufs=2) as pool_sa,
            tc.tile_pool(name="sT", bufs=2) as pool_st,
            tc.tile_pool(name="ro", bufs=2) as pool_ro,
            tc.tile_pool(name="ro2", bufs=2) as pool_ro2,
            tc.tile_pool(name="u", bufs=2) as pool_u,
            tc.tile_pool(name="t2t", bufs=4) as pool_t2,
            tc.tile_pool(name="ot", bufs=2) as pool_ot,
            tc.tile_pool(name="pa", bufs=4, space="PSUM") as pool_pa,
            tc.tile_pool(name="pT", bufs=1, space="PSUM") as pool_pt,
            tc.tile_pool(name="pw", bufs=2, space="PSUM") as pool_pw,
            tc.tile_pool(name="pTb", bufs=1, space="PSUM") as pool_ptb,
            tc.tile_pool(name="dram", bufs=1, space="DRAM") as dram,
        ):
            # constants (Pool/SWDGE queue: keeps HWDGE free for X loads)
            w1t = consts.tile([128, 128], bf16, tag="w1")
            nc.gpsimd.dma_start(w1t[:], w1_ext[:])
            w2t = consts.tile([128, 128], bf16, tag="w2")
            nc.gpsimd.dma_start(w2t[:], w2_ext[:])
            b1t = consts.tile([128, 1], f32, tag="b1")
            nc.gpsimd.dma_start(b1t[:], b1_ext[:])
            b2r = consts.tile([128, F], f32, tag="b2r")
            nc.gpsimd.dma_start(b2r[:], b2_ext[:])
            idt = consts.tile([128, 128], bf16, tag="id")
            nc.gpsimd.dma_start(idt[:], id_ext[:])

            # M resident, 4 j2-range tiles per dst block: [128, j2, a, dst].
            # Finer tiles interleave with quad loads on the DMA FIFO and
            # let the A wavefront start after a 0.9us piece, not 3.6us.
            MP = 4
            MJ = NB2 // MP
            mres = [[pool_m.tile([128, MJ, 2, 128], fp8, tag="m",
                                 name=f"m{bi}_{p}") for p in range(MP)]
                    for bi in range(BPC)]
            m_loaded = [[False] * MP for _ in range(BPC)]

            def load_m_part(bi, p, eng=None):
                if not m_loaded[bi][p]:
                    (eng or nc.sync).dma_start(
                        mres[bi][p][:].rearrange("p a b q -> p (a b q)"),
                        m_ext[bi, :, p * MJ * 256:(p + 1) * MJ * 256])
                    m_loaded[bi][p] = True

            def load_m(bi, eng=None):
                for p in range(MP):
                    load_m_part(bi, p, eng)

            # DRAM intermediates (per chain)
            t2loc = [dram.tile([BPC * 128, CW], fp8, tag=f"t2loc{c}",
                               name=f"t2loc{c}")
                     for c in range(NCH)]
            if with_collective:
                t2full = [dram.tile([NP, CW], fp8, tag=f"t2full{c}",
                                    name=f"t2full{c}", addr_space="Shared")
                          for c in range(NCH)]
            else:
                # per-tile pieces: fine-grained comm->load dependencies in
                # the timing surrogate (same total receive traffic)
                t2full = [[dram.tile([TB * 128, CW], fp8, tag=f"t2f{c}_{q}",
                                     name=f"t2f{c}_{q}")
                           for q in range(NT)] for c in range(NCH)]

            def new_set(label):
                return [pool_set.tile([128, TB, CW], fp8, tag="set",
                                      name=f"{label}_q{q}")
                        for q in range(NT)]

            def x_load(c, st, q):
                nc.sync.dma_start(
                    st[q][:],
                    x_ext[q * TB:(q + 1) * TB, :, c * CW:(c + 1) * CW]
                    .rearrange("a p f -> p a f"))

            def t2c_load(c, st, q):
                if with_collective:
                    src = (t2full[c][q * TB * 128:(q + 1) * TB * 128, :]
                           .rearrange("(a p) f -> p a f", p=128))
                else:
                    src = t2full[c][q][:].rearrange("(a p) f -> p a f",
                                                    p=128)
                nc.sync.dma_start(st[q][:], src)

            def comm_write(c, r):
                """r-th piece of the chain-c AllGather, emitted spread out
                so its DMA traffic interleaves with compute-window DMA."""
                if with_collective:
                    if r == 0:
                        nc.gpsimd.collective_compute(
                            "AllGather", mybir.AluOpType.bypass,
                            replica_groups=[list(range(N_CORES))],
                            ins=[t2loc[c][:]], outs=[t2full[c][:]])
                else:
                    # timing stand-in: emulate receive traffic, per piece
                    # (timing-only; contents unchecked in this build)
                    nc.gpsimd.dma_start(t2full[c][r][:],
                                        t2loc[c][0:TB * 128, :])

            NCW = 1 if with_collective else NT  # comm pieces per chain

            def a_mms(st, bi):
                """psum [128, CW] = sum_j2 M[bi,j2] @ st[j2]."""
                ps = pool_pa.tile([128, CW], f32, tag="pa")
                h = TB // 2  # j2 pairs per set tile
                for j2 in range(NB2):
                    nc.tensor.matmul(
                        ps[:], mres[bi][j2 // MJ][:, j2 % MJ],
                        st[j2 // h][:, 2 * (j2 % h):2 * (j2 % h) + 2, :],
                        start=(j2 == 0), stop=(j2 == NB2 - 1),
                        perf_mode=DR)
                return ps

            def chain_stage(c, layer, st, extra=None, fill=False,
                            last=False):
                """One chain of one layer, software-pipelined across bi.
                Per step: PE runs [A(bi), Wmm(bi-2), T(bi-1), Tback(bi-3)];
                DVE/Act run the psum<->sbuf copies a step behind.
                fill=True: the first 4 dst blocks accumulate j2-major so PE
                tracks the arriving X tiles instead of idling (pipeline
                fill at program start)."""
                sAs, sTs, ros, ro2s, pas = {}, {}, {}, {}, {}

                pss = {}
                if fill:
                    NF = 4
                    h = TB // 2
                    for bi in range(NF):
                        pss[bi] = pool_pa.tile([128, CW], f32, tag="pa",
                                               name=f"paf{bi}")
                    # diagonal (tile+bi) order: each accumulator advances as
                    # soon as its X tile and M piece have landed
                    for s in range(NT + NF - 1):
                        for bi in range(NF):
                            q = s - bi
                            if not 0 <= q < NT:
                                continue
                            for j2 in range(q * h, (q + 1) * h):
                                nc.tensor.matmul(
                                    pss[bi][:], mres[bi][j2 // MJ][:, j2 % MJ],
                                    st[q][:, 2 * (j2 % h):2 * (j2 % h) + 2, :],
                                    start=(j2 == 0), stop=(j2 == NB2 - 1),
                                    perf_mode=DR)

                def aux(i, dst, src):
                    if i % 2 == 0:
                        nc.vector.tensor_scalar_mul(dst, src, 1.0)
                    else:
                        nc.scalar.activation(dst, src, AF.Copy)

                def s_t1(bi):  # transposes of sA(bi) + copy to sbuf
                    pT = pool_pt.tile([128, CW], bf16, tag="pT")
                    for pl in range(4):
                        nc.tensor.transpose(
                            pT[:, pl * 128:(pl + 1) * 128],
                            sAs[bi][:, pl * 128:(pl + 1) * 128], idt[:])
                    sT = pool_st.tile([128, CW], bf16, tag="sT")
                    aux(bi + 1, sT[:], pT[:])
                    sTs[bi] = sT

                def s_w(bi):  # W1 matmul + relu (feature-major)
                    ps = pool_pw.tile([128, CW], f32, tag="pw")
                    nc.tensor.matmul(ps[:], w1t[:], sTs[bi][:],
                                     start=True, stop=True)
                    ro = pool_ro.tile([128, CW], bf16, tag="ro")
                    nc.scalar.activation(ro[:], ps[:], AF.Relu,
                                         scale=0.125, bias=b1t[:])
                    ros[bi] = ro

                def s_w2(bi):  # W2 matmul while still feature-major
                    ps = pool_pw.tile([128, CW], f32, tag="pw")
                    nc.tensor.matmul(ps[:], w2t[:], ros[bi][:],
                                     start=True, stop=True)
                    ro2 = pool_ro2.tile([128, CW], bf16, tag="ro2")
                    aux(bi, ro2[:], ps[:])
                    ro2s[bi] = ro2

                def s_tb(bi):  # transpose tw back to node-major + store
                    pb = pool_ptb.tile([128, CW], bf16, tag="pTb")
                    for pl in range(4):
                        nc.tensor.transpose(
                            pb[:, pl * 128:(pl + 1) * 128],
                            ro2s[bi][:, pl * 128:(pl + 1) * 128], idt[:])
                    t2t = pool_t2.tile([128, CW], fp8, tag="t2t")
                    aux(bi + 1, t2t[:], pb[:])
                    nc.gpsimd.dma_start(
                        t2loc[c][bi * 128:(bi + 1) * 128, :], t2t[:])

                def backend2(bi):  # layer 2: bias + sigmoid + out, dst-major
                    u = pool_u.tile([128, CW], f32, tag="u")
                    nc.vector.scalar_tensor_tensor(
                        u[:], pas[bi][:], 0.125,
                        b2r[:, c * CW:(c + 1) * CW],
                        mybir.AluOpType.mult, mybir.AluOpType.add)
                    del pas[bi]
                    ot = pool_ot.tile([128, CW], f32, tag="ot")
                    nc.scalar.activation(ot[:], u[:], AF.Sigmoid)
                    nc.gpsimd.dma_start(
                        out_ext[bi * 128:(bi + 1) * 128,
                                c * CW:(c + 1) * CW], ot[:])

                if layer == 2:
                    for step in range(BPC + 1):
                        if step < BPC:
                            ps = pss.get(step)
                            if ps is None:
                                ps = a_mms(st, step)
                            pas[step] = ps
                        if step >= 1:
                            backend2(step - 1)
                        if extra is not None:
                            extra(step)
                    return

                depth = 5
                for step in range(BPC + depth):
                    if step < BPC:
                        ps = pss.get(step)
                        if ps is None:
                            ps = a_mms(st, step)
                        sA = pool_sa.tile([128, CW], bf16, tag="sA")
                        aux(step, sA[:], ps[:])
                        sAs[step] = sA
                    if 2 <= step < BPC + 2:
                        s_w(step - 2)
                    if 1 <= step < BPC + 1:
                        s_t1(step - 1)
                    if 3 <= step < BPC + 3:
                        s_w2(step - 3)
                    if 4 <= step < BPC + 4:
                        s_tb(step - 4)
                    if extra is not None:
                        extra(step)

            # ---- pipeline schedule ----
            # fill: chain-0 X tiles stream in (the A wavefront follows
            # them); M0/M1 slot into the stream early, the rest spread
            # one per pipeline step so they never starve the quad loads
            # fill: X tiles stream in with the M parts the j2-major fill
            # blocks (bi 0-3) need, paced piece-wise so neither starves
            st0 = new_set("x0")
            for q in range(NT):
                x_load(0, st0, q)
                # one M piece per X tile: piece k=(p,bi) in bi-major order
                for k in ([q] if q < 8 else [8 + 2 * (q - 8),
                                             9 + 2 * (q - 8)]):
                    if k < 16:
                        load_m_part(k % 4, k // 4, nc.scalar)
            for bi in range(4):
                load_m_part(bi, 3, nc.scalar)

            st1 = new_set("x1")

            def fx1(step):
                # SP queue: 1:1 interleave with next chain's X paces the
                # M streams without blocking any aux-op queue
                if step < NT:
                    x_load(1, st1, step)
                if step + 4 < BPC:
                    load_m(step + 4)
            chain_stage(0, 1, st0, fx1, fill=True)

            def fc0(step):
                if step < NCW:
                    comm_write(0, step)
            chain_stage(1, 1, st1, fc0)

            stl2_0 = new_set("t2c0")
            for q in range(NT):
                t2c_load(0, stl2_0, q)

            st2 = new_set("x2")

            def f20(step):
                if step < NT:
                    x_load(2, st2, step)
                if step < NCW:
                    comm_write(1, step)
            chain_stage(0, 2, stl2_0, f20)

            stl2_1 = new_set("t2c1")

            def f12(step):
                if step < NT:
                    t2c_load(1, stl2_1, step)
            chain_stage(2, 1, st2, f12)

            def f21(step):
                if step < NCW:
                    comm_write(2, step)
            chain_stage(1, 2, stl2_1, f21)

            stl2_2 = new_set("t2c2")
            for q in range(NT):
                t2c_load(2, stl2_2, q)
            chain_stage(2, 2, stl2_2, last=True)

    nc.compile()
    return nc


def prepare_inputs(X, edge_index, W1, b1, W2, b2):
    """Host-side graph/layout prep. Returns per-core in_maps."""
    X = np.asarray(X, dtype=np.float32)
    edge_index = np.asarray(edge_index)
    W1 = np.asarray(W1, dtype=np.float32)
    b1 = np.asarray(b1, dtype=np.float32)
    W2 = np.asarray(W2, dtype=np.float32)
    b2 = np.asarray(b2, dtype=np.float32)

    src = edge_index[0].astype(np.int64)
    dst = edge_index[1].astype(np.int64)

    deg = np.bincount(dst, minlength=N).astype(np.float32) + 1.0
    dinv = 1.0 / np.sqrt(deg)
    dinv_pad = np.zeros(NP, np.float32)
    dinv_pad[:N] = dinv

    # M = 8 * D^-1/2 (Adj + I) D^-1/2; x8 keeps coefficients in fp8e4
    # normal range (compensated by activation scale=1/8)
    Mfull = np.zeros((NP, NP), np.float32)
    np.add.at(Mfull, (dst, src), 1.0)
    Mfull[np.arange(N), np.arange(N)] += 1.0
    Mfull *= 8.0 * dinv_pad[:, None] * dinv_pad[None, :]

    # XN: [sb][p][col], col = s*64+cin with s = b*T+t, raw X in fp8
    XT = np.zeros((NP, F), np.float32)
    XT[:N] = np.transpose(X, (1, 0, 2, 3)).reshape(N, F)
    XN = XT.reshape(NB, 128, F).astype(ml_dtypes.float8_e4m3)

    def blockdiag(W):
        D = np.zeros((128, 128), np.float32)
        D[:64, :64] = W
        D[64:, 64:] = W
        return D.astype(ml_dtypes.bfloat16)

    W1D = blockdiag(W1)
    W2D = blockdiag(W2)
    B1 = np.concatenate([b1, b1])[:, None].astype(np.float32)
    B2R = np.tile(b2, (128, S)).astype(np.float32)
    IDT = np.eye(128, dtype=ml_dtypes.bfloat16)

    in_maps = []
    for c in range(N_CORES):
        rows = Mfull[c * BPC * 128:(c + 1) * BPC * 128, :]
        Mc = rows.reshape(BPC, 128, NB, 128).transpose(0, 3, 2, 1)
        Mc = np.ascontiguousarray(Mc).reshape(BPC, 128, NB * 128)
        Mc = Mc.astype(ml_dtypes.float8_e4m3)
        in_maps.append({"XN": XN, "M": Mc, "W1D": W1D, "W2D": W2D,
                        "B1": B1, "B2R": B2R, "IDT": IDT})
    return in_maps


_NC_CACHE = {}


def kernel(X, edge_index, W1, b1, W2, b2):
    if "nc" not in _NC_CACHE:
        _NC_CACHE["nc"] = build_program(with_collective=True)
    nc = _NC_CACHE["nc"]
    in_maps = prepare_inputs(X, edge_index, W1, b1, W2, b2)

    res = None
    for attempt in range(5):
        try:
            res = run_bass_kernel_spmd(nc, in_maps, list(range(N_CORES)))
            break
        except Exception:
            if attempt == 4:
                raise
            time.sleep(60.0 * (attempt + 1))
    assert res is not None

    # reassemble: per core [1280, 24*64] node-major -> [24, 64, 1280]
    full = np.zeros((S, C, N), np.float32)
    for c in range(N_CORES):
        o = res.results[c]["OUT"].reshape(BPC * 128, S, C)
        lo = c * BPC * 128
        hi = min(N, (c + 1) * BPC * 128)
        if lo < N:
            full[:, :, lo:hi] = o[:hi - lo].transpose(1, 2, 0)
    out = full.reshape(B, T, C, N).transpose(0, 3, 1, 2)
    return np.ascontiguousarray(out)


# revision 64
# speedup vs baseline: 1.0427x; 1.0124x over previous
"""GCN block (2-layer) Trainium2 Bass kernel.

Math (per B*T slice, shared graph):
  t2 = relu(A @ X @ W1 + b1);  out = sigmoid(A @ t2 @ W2 + b2)
  A = D^-1/2 (Adj + I) D^-1/2  (PyG gcn_norm, counts edge multiplicity)

Device mapping:
  A is applied as dense 128x128 blocks of M = 8*A in fp8e4 (x8 keeps
  every nonzero coefficient in fp8-normal range; the 1/8 is compensated
  exactly downstream) via PE matmuls in fp8 DoubleRow mode (K = 256 src
  nodes per matmul) accumulating in PSUM. The moving operand is X itself
  (layer 1) or tw (layer 2), node-major fp8 — no separate X@W1
  front-end exists. Layer 1's per-dst-block epilogue computes, while
  feature-major: PE-transpose 128x128 pair-blocks (identity matmul) ->
  W1 matmul (blockdiag, bf16) -> relu(x/8 + b1) -> W2 matmul -> back-
  transpose to node-major -> tw fp8. Layer 2 is then just the A stage
  plus a single sigmoid(x/8) straight from PSUM, writing dst-major OUT
  directly (the host transposes at assembly); b2 rides along as a
  virtual bias node (tw row 10000 = b2, M column 10000 = 8).

Sharding: each of 8 cores owns 10 of the 80 dst-node blocks (128 nodes
each, N padded 10000->10240) for ALL 24 B*T slices; the relu'd layer-1
activations are AllGathered between the layers in three 512-column
chain chunks so comm overlaps compute.

Layout: the 1536 free columns (24 slices x 64 features, col = s*64+f)
are processed in 3 chains of 512. M stays SBUF-resident (loaded once);
the moving X/t2 operands live in per-chain sets of 10 tiles
[128, 8 node blocks, 512] fp8, double-buffered (2 sets), so A-stages
start on the first node blocks while later ones still stream in.
Each chain stage is software-pipelined: per step the PE runs
[A-matmuls(bi), W-matmul(bi-2), transposes(bi-1), back-transposes]
so it never waits on the psum<->sbuf copies (DVE/Act).
"""
import time

import numpy as np
import ml_dtypes

import concourse.bacc as bacc
import concourse.mybir as mybir
import concourse.tile as tile
from concourse.bass_utils import run_bass_kernel_spmd

N_CORES = 8
N = 10000
NP = 10240            # padded nodes
NB = NP // 128        # 80 node blocks
NB2 = NB // 2         # 40 src-block pairs (DoubleRow K=256)
NT = 10               # tiles per set
TB = NB // NT         # 8 node blocks per set tile
BPC = NB // N_CORES   # 10 dst blocks per core
B, T, C = 2, 12, 64
S = B * T             # 24 slices
F = S * C             # 1536 free columns
PAIRS = S // 2        # 12 slice pairs
NCH = 3               # chains (column chunks)
CW = F // NCH         # 512 cols per chain

f32 = mybir.dt.float32
bf16 = mybir.dt.bfloat16
fp8 = mybir.dt.float8e4
DR = mybir.MatmulPerfMode.DoubleRow
AF = mybir.ActivationFunctionType


def build_program(with_collective=True, nc_hook=None):
    nc = bacc.Bacc("TRN2", target_bir_lowering=False, debug=False,
                   num_devices=N_CORES)
    if nc_hook is not None:
        nc_hook(nc)

    # X node-major: [sb][p][col], col = s*64+cin, fp8 (raw X, no folds)
    x_ext = nc.dram_tensor("XN", [NB, 128, F], fp8, kind="ExternalInput")
    # M rows: [bi][p_src][(j2,a)*128+dst], fp8 = 8*A (both dinv folded)
    m_ext = nc.dram_tensor("M", [BPC, 128, NB * 128], fp8,
                           kind="ExternalInput")
    w1_ext = nc.dram_tensor("W1D", [128, 128], bf16, kind="ExternalInput")
    w2_ext = nc.dram_tensor("W2D", [128, 128], bf16, kind="ExternalInput")
    b1_ext = nc.dram_tensor("B1", [128, 1], f32, kind="ExternalInput")
    b2_ext = nc.dram_tensor("B2Q", [1, F], fp8, kind="ExternalInput")
    id_ext = nc.dram_tensor("IDT", [128, 128], bf16, kind="ExternalInput")
    out_ext = nc.dram_tensor("OUT", [BPC * 128, F], f32,
                             kind="ExternalOutput")

    with tile.TileContext(nc) as tc:
        with (
            tc.tile_pool(name="consts", bufs=1) as consts,
            tc.tile_pool(name="mres", bufs=4 * BPC) as pool_m,
            tc.tile_pool(name="set", bufs=2 * NT) as pool_set,
            tc.tile_pool(name="sA", bufs=2) as pool_sa,
            tc.tile_pool(name="sT", bufs=2) as pool_st,
            tc.tile_pool(name="ro", bufs=2) as pool_ro,
            tc.tile_pool(name="ro2", bufs=2) as pool_ro2,
            tc.tile_pool(name="t2t", bufs=4) as pool_t2,
            tc.tile_pool(name="ot", bufs=2) as pool_ot,
            tc.tile_pool(name="pa", bufs=4, space="PSUM") as pool_pa,
            tc.tile_pool(name="pT", bufs=1, space="PSUM") as pool_pt,
            tc.tile_pool(name="pw", bufs=2, space="PSUM") as pool_pw,
            tc.tile_pool(name="pTb", bufs=1, space="PSUM") as pool_ptb,
            tc.tile_pool(name="dram", bufs=1, space="DRAM") as dram,
        ):
            # constants (Pool/SWDGE queue: keeps HWDGE free for X loads)
            w1t = consts.tile([128, 128], bf16, tag="w1")
            nc.gpsimd.dma_start(w1t[:], w1_ext[:])
            w2t = consts.tile([128, 128], bf16, tag="w2")
            nc.gpsimd.dma_start(w2t[:], w2_ext[:])
            b1t = consts.tile([128, 1], f32, tag="b1")
            nc.gpsimd.dma_start(b1t[:], b1_ext[:])
            idt = consts.tile([128, 128], bf16, tag="id")
            nc.gpsimd.dma_start(idt[:], id_ext[:])

            # M resident, 4 j2-range tiles per dst block: [128, j2, a, dst].
            # Finer tiles interleave with quad loads on the DMA FIFO and
            # let the A wavefront start after a 0.9us piece, not 3.6us.
            MP = 4
            MJ = NB2 // MP
            mres = [[pool_m.tile([128, MJ, 2, 128], fp8, tag="m",
                                 name=f"m{bi}_{p}") for p in range(MP)]
                    for bi in range(BPC)]
            m_loaded = [[False] * MP for _ in range(BPC)]

            def load_m_part(bi, p, eng=None):
                if not m_loaded[bi][p]:
                    (eng or nc.sync).dma_start(
                        mres[bi][p][:].rearrange("p a b q -> p (a b q)"),
                        m_ext[bi, :, p * MJ * 256:(p + 1) * MJ * 256])
                    m_loaded[bi][p] = True

            def load_m(bi, eng=None):
                for p in range(MP):
                    load_m_part(bi, p, eng)

            # DRAM intermediates (per chain)
            t2loc = [dram.tile([BPC * 128, CW], fp8, tag=f"t2loc{c}",
                               name=f"t2loc{c}")
                     for c in range(NCH)]
            if with_collective:
                t2full = [dram.tile([NP, CW], fp8, tag=f"t2full{c}",
                                    name=f"t2full{c}", addr_space="Shared")
                          for c in range(NCH)]
            else:
                # per-tile pieces: fine-grained comm->load dependencies in
                # the timing surrogate (same total receive traffic)
                t2full = [[dram.tile([TB * 128, CW], fp8, tag=f"t2f{c}_{q}",
                                     name=f"t2f{c}_{q}")
                           for q in range(NT)] for c in range(NCH)]

            def new_set(label):
                return [pool_set.tile([128, TB, CW], fp8, tag="set",
                                      name=f"{label}_q{q}")
                        for q in range(NT)]

            def x_load(c, st, q):
                nc.sync.dma_start(
                    st[q][:],
                    x_ext[q * TB:(q + 1) * TB, :, c * CW:(c + 1) * CW]
                    .rearrange("a p f -> p a f"))

            def t2c_load(c, st, q):
                """Load gathered-t2 tile q; tile 9 gets the bias node:
                row 10000 (graph padding) overwritten with b2, which with
                M[:,10000]=8 and the 1/8 sigmoid scale acts as "+ b2"."""
                if with_collective:
                    src = (t2full[c][q * TB * 128:(q + 1) * TB * 128, :]
                           .rearrange("(a p) f -> p a f", p=128))
                else:
                    src = t2full[c][q][:].rearrange("(a p) f -> p a f",
                                                    p=128)
                nc.sync.dma_start(st[q][:], src)
                if q == NT - 1:
                    sb, p = 10000 // 128, 10000 % 128
                    nc.sync.dma_start(
                        st[q][p:p + 1, sb % TB, :],
                        b2_ext[0:1, c * CW:(c + 1) * CW])

            def comm_write(c, r):
                """r-th piece of the chain-c AllGather, emitted spread out
                so its DMA traffic interleaves with compute-window DMA."""
                if with_collective:
                    if r == 0:
                        nc.gpsimd.collective_compute(
                            "AllGather", mybir.AluOpType.bypass,
                            replica_groups=[list(range(N_CORES))],
                            ins=[t2loc[c][:]], outs=[t2full[c][:]])
                else:
                    # timing stand-in: emulate receive traffic, per piece
                    # (timing-only; contents unchecked in this build)
                    nc.gpsimd.dma_start(t2full[c][r][:],
                                        t2loc[c][0:TB * 128, :])

            NCW = 1 if with_collective else NT  # comm pieces per chain

            def a_mms(st, bi):
                """psum [128, CW] = sum_j2 M[bi,j2] @ st[j2]."""
                ps = pool_pa.tile([128, CW], f32, tag="pa")
                h = TB // 2  # j2 pairs per set tile
                for j2 in range(NB2):
                    nc.tensor.matmul(
                        ps[:], mres[bi][j2 // MJ][:, j2 % MJ],
                        st[j2 // h][:, 2 * (j2 % h):2 * (j2 % h) + 2, :],
                        start=(j2 == 0), stop=(j2 == NB2 - 1),
                        perf_mode=DR)
                return ps

            def chain_stage(c, layer, st, extra=None, fill=False,
                            last=False):
                """One chain of one layer, software-pipelined across bi.
                Per step: PE runs [A(bi), Wmm(bi-2), T(bi-1), Tback(bi-3)];
                DVE/Act run the psum<->sbuf copies a step behind.
                fill=True: the first 4 dst blocks accumulate j2-major so PE
                tracks the arriving X tiles instead of idling (pipeline
                fill at program start)."""
                sAs, sTs, ros, ro2s, pas = {}, {}, {}, {}, {}

                pss = {}
                if fill:
                    NF = 4
                    h = TB // 2
                    for bi in range(NF):
                        pss[bi] = pool_pa.tile([128, CW], f32, tag="pa",
                                               name=f"paf{bi}")
                    # diagonal (tile+bi) order: each accumulator advances as
                    # soon as its X tile and M piece have landed
                    for s in range(NT + NF - 1):
                        for bi in range(NF):
                            q = s - bi
                            if not 0 <= q < NT:
                                continue
                            for j2 in range(q * h, (q + 1) * h):
                                nc.tensor.matmul(
                                    pss[bi][:], mres[bi][j2 // MJ][:, j2 % MJ],
                                    st[q][:, 2 * (j2 % h):2 * (j2 % h) + 2, :],
                                    start=(j2 == 0), stop=(j2 == NB2 - 1),
                                    perf_mode=DR)

                def aux(i, dst, src):
                    if i % 2 == 0:
                        nc.vector.tensor_scalar_mul(dst, src, 1.0)
                    else:
                        nc.scalar.activation(dst, src, AF.Copy)

                def s_t1(bi):  # transposes of sA(bi) + copy to sbuf
                    pT = pool_pt.tile([128, CW], bf16, tag="pT")
                    for pl in range(4):
                        nc.tensor.transpose(
                            pT[:, pl * 128:(pl + 1) * 128],
                            sAs[bi][:, pl * 128:(pl + 1) * 128], idt[:])
                    sT = pool_st.tile([128, CW], bf16, tag="sT")
                    aux(bi + 1, sT[:], pT[:])
                    sTs[bi] = sT

                def s_w(bi):  # W1 matmul + relu (feature-major)
                    ps = pool_pw.tile([128, CW], f32, tag="pw")
                    nc.tensor.matmul(ps[:], w1t[:], sTs[bi][:],
                                     start=True, stop=True)
                    ro = pool_ro.tile([128, CW], bf16, tag="ro")
                    nc.scalar.activation(ro[:], ps[:], AF.Relu,
                                         scale=0.125, bias=b1t[:])
                    ros[bi] = ro

                def s_w2(bi):  # W2 matmul while still feature-major
                    ps = pool_pw.tile([128, CW], f32, tag="pw")
                    nc.tensor.matmul(ps[:], w2t[:], ros[bi][:],
                                     start=True, stop=True)
                    ro2 = pool_ro2.tile([128, CW], bf16, tag="ro2")
                    aux(bi, ro2[:], ps[:])
                    ro2s[bi] = ro2

                def s_tb(bi):  # transpose tw back to node-major + store
                    pb = pool_ptb.tile([128, CW], bf16, tag="pTb")
                    for pl in range(4):
                        nc.tensor.transpose(
                            pb[:, pl * 128:(pl + 1) * 128],
                            ro2s[bi][:, pl * 128:(pl + 1) * 128], idt[:])
                    t2t = pool_t2.tile([128, CW], fp8, tag="t2t")
                    aux(bi + 1, t2t[:], pb[:])
                    nc.gpsimd.dma_start(
                        t2loc[c][bi * 128:(bi + 1) * 128, :], t2t[:])

                def backend2(bi):  # layer 2: sigmoid + out, dst-major
                    ot = pool_ot.tile([128, CW], f32, tag="ot")
                    nc.scalar.activation(ot[:], pas[bi][:], AF.Sigmoid,
                                         scale=0.125)
                    del pas[bi]
                    # SP/HWDGE: shorter issue latency than SWDGE and SP is
                    # otherwise idle here; matters for the final write
                    nc.sync.dma_start(
                        out_ext[bi * 128:(bi + 1) * 128,
                                c * CW:(c + 1) * CW], ot[:])

                if layer == 2:
                    for step in range(BPC + 1):
                        if step < BPC:
                            ps = pss.get(step)
                            if ps is None:
                                ps = a_mms(st, step)
                            pas[step] = ps
                        if step >= 1:
                            backend2(step - 1)
                        if extra is not None:
                            extra(step)
                    return

                depth = 5
                for step in range(BPC + depth):
                    if step < BPC:
                        ps = pss.get(step)
                        if ps is None:
                            ps = a_mms(st, step)
                        sA = pool_sa.tile([128, CW], bf16, tag="sA")
                        aux(step, sA[:], ps[:])
                        sAs[step] = sA
                    if 2 <= step < BPC + 2:
                        s_w(step - 2)
                    if 1 <= step < BPC + 1:
                        s_t1(step - 1)
                    if 3 <= step < BPC + 3:
                        s_w2(step - 3)
                    if 4 <= step < BPC + 4:
                        s_tb(step - 4)
                    if extra is not None:
                        extra(step)

            # ---- pipeline schedule ----
            # fill: chain-0 X tiles stream in (the A wavefront follows
            # them); M0/M1 slot into the stream early, the rest spread
            # one per pipeline step so they never starve the quad loads
            # fill: X tiles stream in with the M parts the j2-major fill
            # blocks (bi 0-3) need, paced piece-wise so neither starves
            st0 = new_set("x0")
            for q in range(NT):
                x_load(0, st0, q)
                # one M piece per X tile: piece k=(p,bi) in bi-major order
                for k in ([q] if q < 8 else [8 + 2 * (q - 8),
                                             9 + 2 * (q - 8)]):
                    if k < 16:
                        load_m_part(k % 4, k // 4, nc.scalar)
            for bi in range(4):
                load_m_part(bi, 3, nc.scalar)

            st1 = new_set("x1")

            def fx1(step):
                # SP queue: 1:1 interleave with next chain's X paces the
                # M streams without blocking any aux-op queue
                if step < NT:
                    x_load(1, st1, step)
                if step + 4 < BPC:
                    load_m(step + 4)
            chain_stage(0, 1, st0, fx1, fill=True)

            def fc0(step):
                if step < NCW:
                    comm_write(0, step)
            chain_stage(1, 1, st1, fc0)

            stl2_0 = new_set("t2c0")
            for q in range(NT):
                t2c_load(0, stl2_0, q)

            st2 = new_set("x2")

            def f20(step):
                if step < NT:
                    x_load(2, st2, step)
                if step < NCW:
                    comm_write(1, step)
            chain_stage(0, 2, stl2_0, f20)

            stl2_1 = new_set("t2c1")

            def f12(step):
                if step < NT:
                    t2c_load(1, stl2_1, step)
            chain_stage(2, 1, st2, f12)

            def f21(step):
                if step < NCW:
                    comm_write(2, step)
            chain_stage(1, 2, stl2_1, f21)

            stl2_2 = new_set("t2c2")
            for q in range(NT):
                t2c_load(2, stl2_2, q)
            chain_stage(2, 2, stl2_2, last=True)

    nc.compile()
    return nc


def prepare_inputs(X, edge_index, W1, b1, W2, b2):
    """Host-side graph/layout prep. Returns per-core in_maps."""
    X = np.asarray(X, dtype=np.float32)
    edge_index = np.asarray(edge_index)
    W1 = np.asarray(W1, dtype=np.float32)
    b1 = np.asarray(b1, dtype=np.float32)
    W2 = np.asarray(W2, dtype=np.float32)
    b2 = np.asarray(b2, dtype=np.float32)

    src = edge_index[0].astype(np.int64)
    dst = edge_index[1].astype(np.int64)

    deg = np.bincount(dst, minlength=N).astype(np.float32) + 1.0
    dinv = 1.0 / np.sqrt(deg)
    dinv_pad = np.zeros(NP, np.float32)
    dinv_pad[:N] = dinv

    # M = 8 * D^-1/2 (Adj + I) D^-1/2; x8 keeps coefficients in fp8e4
    # normal range (compensated by activation scale=1/8)
    Mfull = np.zeros((NP, NP), np.float32)
    np.add.at(Mfull, (dst, src), 1.0)
    Mfull[np.arange(N), np.arange(N)] += 1.0
    Mfull *= 8.0 * dinv_pad[:, None] * dinv_pad[None, :]
    Mfull[:, 10000] = 8.0  # bias column (see comm_write inject)

    # XN: [sb][p][col], col = s*64+cin with s = b*T+t, raw X in fp8
    XT = np.zeros((NP, F), np.float32)
    XT[:N] = np.transpose(X, (1, 0, 2, 3)).reshape(N, F)
    XN = XT.reshape(NB, 128, F).astype(ml_dtypes.float8_e4m3)

    def blockdiag(W):
        D = np.zeros((128, 128), np.float32)
        D[:64, :64] = W
        D[64:, 64:] = W
        return D.astype(ml_dtypes.bfloat16)

    W1D = blockdiag(W1)
    W2D = blockdiag(W2)
    B1 = np.concatenate([b1, b1])[:, None].astype(np.float32)
    B2Q = np.tile(b2, (1, S)).astype(ml_dtypes.float8_e4m3)
    IDT = np.eye(128, dtype=ml_dtypes.bfloat16)

    in_maps = []
    for c in range(N_CORES):
        rows = Mfull[c * BPC * 128:(c + 1) * BPC * 128, :]
        Mc = rows.reshape(BPC, 128, NB, 128).transpose(0, 3, 2, 1)
        Mc = np.ascontiguousarray(Mc).reshape(BPC, 128, NB * 128)
        Mc = Mc.astype(ml_dtypes.float8_e4m3)
        in_maps.append({"XN": XN, "M": Mc, "W1D": W1D, "W2D": W2D,
                        "B1": B1, "B2Q": B2Q, "IDT": IDT})
    return in_maps


_NC_CACHE = {}


def kernel(X, edge_index, W1, b1, W2, b2):
    if "nc" not in _NC_CACHE:
        _NC_CACHE["nc"] = build_program(with_collective=True)
    nc = _NC_CACHE["nc"]
    in_maps = prepare_inputs(X, edge_index, W1, b1, W2, b2)

    res = None
    for attempt in range(5):
        try:
            res = run_bass_kernel_spmd(nc, in_maps, list(range(N_CORES)))
            break
        except Exception:
            if attempt == 4:
                raise
            time.sleep(60.0 * (attempt + 1))
    assert res is not None

    # reassemble: per core [1280, 24*64] node-major -> [24, 64, 1280]
    full = np.zeros((S, C, N), np.float32)
    for c in range(N_CORES):
        o = res.results[c]["OUT"].reshape(BPC * 128, S, C)
        lo = c * BPC * 128
        hi = min(N, (c + 1) * BPC * 128)
        if lo < N:
            full[:, :, lo:hi] = o[:hi - lo].transpose(1, 2, 0)
    out = full.reshape(B, T, C, N).transpose(0, 3, 1, 2)
    return np.ascontiguousarray(out)


# revision 65
# speedup vs baseline: 1.0427x; 1.0000x over previous
"""GCN block (2-layer) Trainium2 Bass kernel.

Math (per B*T slice, shared graph):
  t2 = relu(A @ X @ W1 + b1);  out = sigmoid(A @ t2 @ W2 + b2)
  A = D^-1/2 (Adj + I) D^-1/2  (PyG gcn_norm, counts edge multiplicity)

Device mapping:
  A is applied as dense 128x128 blocks of M = 8*A in fp8e4 (x8 keeps
  every nonzero coefficient in fp8-normal range; the 1/8 is compensated
  exactly downstream) via PE matmuls in fp8 DoubleRow mode (K = 256 src
  nodes per matmul) accumulating in PSUM. The moving operand is X itself
  (layer 1) or tw (layer 2), node-major fp8 — no separate X@W1
  front-end exists. Layer 1's per-dst-block epilogue computes, while
  feature-major: PE-transpose 128x128 pair-blocks (identity matmul) ->
  W1 matmul (blockdiag, bf16) -> relu(x/8 + b1) -> W2 matmul -> back-
  transpose to node-major -> tw fp8. Layer 2 is then just the A stage
  plus a single sigmoid(x/8) straight from PSUM, writing dst-major OUT
  directly (the host transposes at assembly); b2 rides along as a
  virtual bias node (tw row 10000 = b2, M column 10000 = 8).

Sharding: each of 8 cores owns 10 of the 80 dst-node blocks (128 nodes
each, N padded 10000->10240) for ALL 24 B*T slices; the relu'd layer-1
activations are AllGathered between the layers in three 512-column
chain chunks so comm overlaps compute.

Layout: the 1536 free columns (24 slices x 64 features, col = s*64+f)
are processed in 3 chains of 512. M stays SBUF-resident (loaded once);
the moving X/t2 operands live in per-chain sets of 10 tiles
[128, 8 node blocks, 512] fp8, double-buffered (2 sets), so A-stages
start on the first node blocks while later ones still stream in.
Each chain stage is software-pipelined: per step the PE runs
[A-matmuls(bi), W-matmul(bi-2), transposes(bi-1), back-transposes]
so it never waits on the psum<->sbuf copies (DVE/Act).
"""
import time

import numpy as np
import ml_dtypes

import concourse.bacc as bacc
import concourse.mybir as mybir
import concourse.tile as tile
from concourse.bass_utils import run_bass_kernel_spmd

N_CORES = 8
N = 10000
NP = 10240            # padded nodes
NB = NP // 128        # 80 node blocks
NB2 = NB // 2         # 40 src-block pairs (DoubleRow K=256)
NT = 10               # tiles per set
TB = NB // NT         # 8 node blocks per set tile
BPC = NB // N_CORES   # 10 dst blocks per core
B, T, C = 2, 12, 64
S = B * T             # 24 slices
F = S * C             # 1536 free columns
PAIRS = S // 2        # 12 slice pairs
NCH = 3               # chains (column chunks)
CW = F // NCH         # 512 cols per chain

f32 = mybir.dt.float32
bf16 = mybir.dt.bfloat16
fp8 = mybir.dt.float8e4
DR = mybir.MatmulPerfMode.DoubleRow
AF = mybir.ActivationFunctionType


def build_program(with_collective=True, nc_hook=None):
    nc = bacc.Bacc("TRN2", target_bir_lowering=False, debug=False,
                   num_devices=N_CORES)
    if nc_hook is not None:
        nc_hook(nc)

    # X node-major: [sb][p][col], col = s*64+cin, fp8 (raw X, no folds)
    x_ext = nc.dram_tensor("XN", [NB, 128, F], fp8, kind="ExternalInput")
    # M rows: [bi][p_src][(j2,a)*128+dst], fp8 = 8*A (both dinv folded)
    m_ext = nc.dram_tensor("M", [BPC, 128, NB * 128], fp8,
                           kind="ExternalInput")
    w1_ext = nc.dram_tensor("W1D", [128, 128], bf16, kind="ExternalInput")
    w2_ext = nc.dram_tensor("W2D", [128, 128], bf16, kind="ExternalInput")
    b1_ext = nc.dram_tensor("B1", [128, 1], f32, kind="ExternalInput")
    b2_ext = nc.dram_tensor("B2Q", [1, F], fp8, kind="ExternalInput")
    id_ext = nc.dram_tensor("IDT", [128, 128], bf16, kind="ExternalInput")
    out_ext = nc.dram_tensor("OUT", [BPC * 128, F], f32,
                             kind="ExternalOutput")

    with tile.TileContext(nc) as tc:
        with (
            tc.tile_pool(name="consts", bufs=1) as consts,
            tc.tile_pool(name="mres", bufs=4 * BPC) as pool_m,
            tc.tile_pool(name="set", bufs=2 * NT) as pool_set,
            tc.tile_pool(name="sA", bufs=3) as pool_sa,
            tc.tile_pool(name="sT", bufs=3) as pool_st,
            tc.tile_pool(name="ro", bufs=3) as pool_ro,
            tc.tile_pool(name="ro2", bufs=3) as pool_ro2,
            tc.tile_pool(name="t2t", bufs=6) as pool_t2,
            tc.tile_pool(name="ot", bufs=3) as pool_ot,
            tc.tile_pool(name="pa", bufs=4, space="PSUM") as pool_pa,
            tc.tile_pool(name="pT", bufs=1, space="PSUM") as pool_pt,
            tc.tile_pool(name="pw", bufs=2, space="PSUM") as pool_pw,
            tc.tile_pool(name="pTb", bufs=1, space="PSUM") as pool_ptb,
            tc.tile_pool(name="dram", bufs=1, space="DRAM") as dram,
        ):
            # constants (Pool/SWDGE queue: keeps HWDGE free for X loads)
            w1t = consts.tile([128, 128], bf16, tag="w1")
            nc.gpsimd.dma_start(w1t[:], w1_ext[:])
            w2t = consts.tile([128, 128], bf16, tag="w2")
            nc.gpsimd.dma_start(w2t[:], w2_ext[:])
            b1t = consts.tile([128, 1], f32, tag="b1")
            nc.gpsimd.dma_start(b1t[:], b1_ext[:])
            idt = consts.tile([128, 128], bf16, tag="id")
            nc.gpsimd.dma_start(idt[:], id_ext[:])

            # M resident, 4 j2-range tiles per dst block: [128, j2, a, dst].
            # Finer tiles interleave with quad loads on the DMA FIFO and
            # let the A wavefront start after a 0.9us piece, not 3.6us.
            MP = 4
            MJ = NB2 // MP
            mres = [[pool_m.tile([128, MJ, 2, 128], fp8, tag="m",
                                 name=f"m{bi}_{p}") for p in range(MP)]
                    for bi in range(BPC)]
            m_loaded = [[False] * MP for _ in range(BPC)]

            def load_m_part(bi, p, eng=None):
                if not m_loaded[bi][p]:
                    (eng or nc.sync).dma_start(
                        mres[bi][p][:].rearrange("p a b q -> p (a b q)"),
                        m_ext[bi, :, p * MJ * 256:(p + 1) * MJ * 256])
                    m_loaded[bi][p] = True

            def load_m(bi, eng=None):
                for p in range(MP):
                    load_m_part(bi, p, eng)

            # DRAM intermediates (per chain)
            t2loc = [dram.tile([BPC * 128, CW], fp8, tag=f"t2loc{c}",
                               name=f"t2loc{c}")
                     for c in range(NCH)]
            if with_collective:
                t2full = [dram.tile([NP, CW], fp8, tag=f"t2full{c}",
                                    name=f"t2full{c}", addr_space="Shared")
                          for c in range(NCH)]
            else:
                # per-tile pieces: fine-grained comm->load dependencies in
                # the timing surrogate (same total receive traffic)
                t2full = [[dram.tile([TB * 128, CW], fp8, tag=f"t2f{c}_{q}",
                                     name=f"t2f{c}_{q}")
                           for q in range(NT)] for c in range(NCH)]

            def new_set(label):
                return [pool_set.tile([128, TB, CW], fp8, tag="set",
                                      name=f"{label}_q{q}")
                        for q in range(NT)]

            def x_load(c, st, q):
                nc.sync.dma_start(
                    st[q][:],
                    x_ext[q * TB:(q + 1) * TB, :, c * CW:(c + 1) * CW]
                    .rearrange("a p f -> p a f"))

            def t2c_load(c, st, q):
                """Load gathered-t2 tile q; tile 9 gets the bias node:
                row 10000 (graph padding) overwritten with b2, which with
                M[:,10000]=8 and the 1/8 sigmoid scale acts as "+ b2"."""
                if with_collective:
                    src = (t2full[c][q * TB * 128:(q + 1) * TB * 128, :]
                           .rearrange("(a p) f -> p a f", p=128))
                else:
                    src = t2full[c][q][:].rearrange("(a p) f -> p a f",
                                                    p=128)
                nc.sync.dma_start(st[q][:], src)
                if q == NT - 1:
                    sb, p = 10000 // 128, 10000 % 128
                    nc.sync.dma_start(
                        st[q][p:p + 1, sb % TB, :],
                        b2_ext[0:1, c * CW:(c + 1) * CW])

            def comm_write(c, r):
                """r-th piece of the chain-c AllGather, emitted spread out
                so its DMA traffic interleaves with compute-window DMA."""
                if with_collective:
                    if r == 0:
                        nc.gpsimd.collective_compute(
                            "AllGather", mybir.AluOpType.bypass,
                            replica_groups=[list(range(N_CORES))],
                            ins=[t2loc[c][:]], outs=[t2full[c][:]])
                else:
                    # timing stand-in: emulate receive traffic, per piece
                    # (timing-only; contents unchecked in this build)
                    nc.gpsimd.dma_start(t2full[c][r][:],
                                        t2loc[c][0:TB * 128, :])

            NCW = 1 if with_collective else NT  # comm pieces per chain

            def a_mms(st, bi):
                """psum [128, CW] = sum_j2 M[bi,j2] @ st[j2]."""
                ps = pool_pa.tile([128, CW], f32, tag="pa")
                h = TB // 2  # j2 pairs per set tile
                for j2 in range(NB2):
                    nc.tensor.matmul(
                        ps[:], mres[bi][j2 // MJ][:, j2 % MJ],
                        st[j2 // h][:, 2 * (j2 % h):2 * (j2 % h) + 2, :],
                        start=(j2 == 0), stop=(j2 == NB2 - 1),
                        perf_mode=DR)
                return ps

            def chain_stage(c, layer, st, extra=None, fill=False,
                            last=False):
                """One chain of one layer, software-pipelined across bi.
                Per step: PE runs [A(bi), Wmm(bi-2), T(bi-1), Tback(bi-3)];
                DVE/Act run the psum<->sbuf copies a step behind.
                fill=True: the first 4 dst blocks accumulate j2-major so PE
                tracks the arriving X tiles instead of idling (pipeline
                fill at program start)."""
                sAs, sTs, ros, ro2s, pas = {}, {}, {}, {}, {}

                pss = {}
                if fill:
                    NF = 4
                    h = TB // 2
                    for bi in range(NF):
                        pss[bi] = pool_pa.tile([128, CW], f32, tag="pa",
                                               name=f"paf{bi}")
                    # diagonal (tile+bi) order: each accumulator advances as
                    # soon as its X tile and M piece have landed
                    for s in range(NT + NF - 1):
                        for bi in range(NF):
                            q = s - bi
                            if not 0 <= q < NT:
                                continue
                            for j2 in range(q * h, (q + 1) * h):
                                nc.tensor.matmul(
                                    pss[bi][:], mres[bi][j2 // MJ][:, j2 % MJ],
                                    st[q][:, 2 * (j2 % h):2 * (j2 % h) + 2, :],
                                    start=(j2 == 0), stop=(j2 == NB2 - 1),
                                    perf_mode=DR)

                def aux(i, dst, src):
                    if i % 2 == 0:
                        nc.vector.tensor_scalar_mul(dst, src, 1.0)
                    else:
                        nc.scalar.activation(dst, src, AF.Copy)

                def s_t1(bi):  # transposes of sA(bi) + copy to sbuf
                    pT = pool_pt.tile([128, CW], bf16, tag="pT")
                    for pl in range(4):
                        nc.tensor.transpose(
                            pT[:, pl * 128:(pl + 1) * 128],
                            sAs[bi][:, pl * 128:(pl + 1) * 128], idt[:])
                    sT = pool_st.tile([128, CW], bf16, tag="sT")
                    aux(bi + 1, sT[:], pT[:])
                    sTs[bi] = sT

                def s_w(bi):  # W1 matmul + relu (feature-major)
                    ps = pool_pw.tile([128, CW], f32, tag="pw")
                    nc.tensor.matmul(ps[:], w1t[:], sTs[bi][:],
                                     start=True, stop=True)
                    ro = pool_ro.tile([128, CW], bf16, tag="ro")
                    nc.scalar.activation(ro[:], ps[:], AF.Relu,
                                         scale=0.125, bias=b1t[:])
                    ros[bi] = ro

                def s_w2(bi):  # W2 matmul while still feature-major
                    ps = pool_pw.tile([128, CW], f32, tag="pw")
                    nc.tensor.matmul(ps[:], w2t[:], ros[bi][:],
                                     start=True, stop=True)
                    ro2 = pool_ro2.tile([128, CW], bf16, tag="ro2")
                    aux(bi, ro2[:], ps[:])
                    ro2s[bi] = ro2

                def s_tb(bi):  # transpose tw back to node-major + store
                    pb = pool_ptb.tile([128, CW], bf16, tag="pTb")
                    for pl in range(4):
                        nc.tensor.transpose(
                            pb[:, pl * 128:(pl + 1) * 128],
                            ro2s[bi][:, pl * 128:(pl + 1) * 128], idt[:])
                    t2t = pool_t2.tile([128, CW], fp8, tag="t2t")
                    aux(bi + 1, t2t[:], pb[:])
                    nc.gpsimd.dma_start(
                        t2loc[c][bi * 128:(bi + 1) * 128, :], t2t[:])

                def backend2(bi):  # layer 2: sigmoid + out, dst-major
                    ot = pool_ot.tile([128, CW], f32, tag="ot")
                    nc.scalar.activation(ot[:], pas[bi][:], AF.Sigmoid,
                                         scale=0.125)
                    del pas[bi]
                    # SP/HWDGE: shorter issue latency than SWDGE and SP is
                    # otherwise idle here; matters for the final write
                    nc.sync.dma_start(
                        out_ext[bi * 128:(bi + 1) * 128,
                                c * CW:(c + 1) * CW], ot[:])

                if layer == 2:
                    for step in range(BPC + 1):
                        if step < BPC:
                            ps = pss.get(step)
                            if ps is None:
                                ps = a_mms(st, step)
                            pas[step] = ps
                        if step >= 1:
                            backend2(step - 1)
                        if extra is not None:
                            extra(step)
                    return

                depth = 5
                for step in range(BPC + depth):
                    if step < BPC:
                        ps = pss.get(step)
                        if ps is None:
                            ps = a_mms(st, step)
                        sA = pool_sa.tile([128, CW], bf16, tag="sA")
                        aux(step, sA[:], ps[:])
                        sAs[step] = sA
                    if 2 <= step < BPC + 2:
                        s_w(step - 2)
                    if 1 <= step < BPC + 1:
                        s_t1(step - 1)
                    if 3 <= step < BPC + 3:
                        s_w2(step - 3)
                    if 4 <= step < BPC + 4:
                        s_tb(step - 4)
                    if extra is not None:
                        extra(step)

            # ---- pipeline schedule ----
            # fill: chain-0 X tiles stream in (the A wavefront follows
            # them); M0/M1 slot into the stream early, the rest spread
            # one per pipeline step so they never starve the quad loads
            # fill: X tiles stream in with the M parts the j2-major fill
            # blocks (bi 0-3) need, paced piece-wise so neither starves
            st0 = new_set("x0")
            for q in range(NT):
                x_load(0, st0, q)
                # one M piece per X tile: piece k=(p,bi) in bi-major order
                for k in ([q] if q < 8 else [8 + 2 * (q - 8),
                                             9 + 2 * (q - 8)]):
                    if k < 16:
                        load_m_part(k % 4, k // 4, nc.scalar)
            for bi in range(4):
                load_m_part(bi, 3, nc.scalar)

            st1 = new_set("x1")

            def fx1(step):
                # SP queue: 1:1 interleave with next chain's X paces the
                # M streams without blocking any aux-op queue
                if step < NT:
                    x_load(1, st1, step)
                if step + 4 < BPC:
                    load_m(step + 4)
            chain_stage(0, 1, st0, fx1, fill=True)

            def fc0(step):
                if step < NCW:
                    comm_write(0, step)
            chain_stage(1, 1, st1, fc0)

            stl2_0 = new_set("t2c0")
            for q in range(NT):
                t2c_load(0, stl2_0, q)

            st2 = new_set("x2")

            def f20(step):
                if step < NT:
                    x_load(2, st2, step)
                if step < NCW:
                    comm_write(1, step)
            chain_stage(0, 2, stl2_0, f20)

            stl2_1 = new_set("t2c1")

            def f12(step):
                if step < NT:
                    t2c_load(1, stl2_1, step)
            chain_stage(2, 1, st2, f12)

            def f21(step):
                if step < NCW:
                    comm_write(2, step)
            chain_stage(1, 2, stl2_1, f21)

            stl2_2 = new_set("t2c2")
            for q in range(NT):
                t2c_load(2, stl2_2, q)
            chain_stage(2, 2, stl2_2, last=True)

    nc.compile()
    return nc


def prepare_inputs(X, edge_index, W1, b1, W2, b2):
    """Host-side graph/layout prep. Returns per-core in_maps."""
    X = np.asarray(X, dtype=np.float32)
    edge_index = np.asarray(edge_index)
    W1 = np.asarray(W1, dtype=np.float32)
    b1 = np.asarray(b1, dtype=np.float32)
    W2 = np.asarray(W2, dtype=np.float32)
    b2 = np.asarray(b2, dtype=np.float32)

    src = edge_index[0].astype(np.int64)
    dst = edge_index[1].astype(np.int64)

    deg = np.bincount(dst, minlength=N).astype(np.float32) + 1.0
    dinv = 1.0 / np.sqrt(deg)
    dinv_pad = np.zeros(NP, np.float32)
    dinv_pad[:N] = dinv

    # M = 8 * D^-1/2 (Adj + I) D^-1/2; x8 keeps coefficients in fp8e4
    # normal range (compensated by activation scale=1/8)
    Mfull = np.zeros((NP, NP), np.float32)
    np.add.at(Mfull, (dst, src), 1.0)
    Mfull[np.arange(N), np.arange(N)] += 1.0
    Mfull *= 8.0 * dinv_pad[:, None] * dinv_pad[None, :]
    Mfull[:, 10000] = 8.0  # bias column (see comm_write inject)

    # XN: [sb][p][col], col = s*64+cin with s = b*T+t, raw X in fp8
    XT = np.zeros((NP, F), np.float32)
    XT[:N] = np.transpose(X, (1, 0, 2, 3)).reshape(N, F)
    XN = XT.reshape(NB, 128, F).astype(ml_dtypes.float8_e4m3)

    def blockdiag(W):
        D = np.zeros((128, 128), np.float32)
        D[:64, :64] = W
        D[64:, 64:] = W
        return D.astype(ml_dtypes.bfloat16)

    W1D = blockdiag(W1)
    W2D = blockdiag(W2)
    B1 = np.concatenate([b1, b1])[:, None].astype(np.float32)
    B2Q = np.tile(b2, (1, S)).astype(ml_dtypes.float8_e4m3)
    IDT = np.eye(128, dtype=ml_dtypes.bfloat16)

    in_maps = []
    for c in range(N_CORES):
        rows = Mfull[c * BPC * 128:(c + 1) * BPC * 128, :]
        Mc = rows.reshape(BPC, 128, NB, 128).transpose(0, 3, 2, 1)
        Mc = np.ascontiguousarray(Mc).reshape(BPC, 128, NB * 128)
        Mc = Mc.astype(ml_dtypes.float8_e4m3)
        in_maps.append({"XN": XN, "M": Mc, "W1D": W1D, "W2D": W2D,
                        "B1": B1, "B2Q": B2Q, "IDT": IDT})
    return in_maps


_NC_CACHE = {}


def kernel(X, edge_index, W1, b1, W2, b2):
    if "nc" not in _NC_CACHE:
        _NC_CACHE["nc"] = build_program(with_collective=True)
    nc = _NC_CACHE["nc"]
    in_maps = prepare_inputs(X, edge_index, W1, b1, W2, b2)

    res = None
    for attempt in range(5):
        try:
            res = run_bass_kernel_spmd(nc, in_maps, list(range(N_CORES)))
            break
        except Exception:
            if attempt == 4:
                raise
            time.sleep(60.0 * (attempt + 1))
    assert res is not None

    # reassemble: per core [1280, 24*64] node-major -> [24, 64, 1280]
    full = np.zeros((S, C, N), np.float32)
    for c in range(N_CORES):
        o = res.results[c]["OUT"].reshape(BPC * 128, S, C)
        lo = c * BPC * 128
        hi = min(N, (c + 1) * BPC * 128)
        if lo < N:
            full[:, :, lo:hi] = o[:hi - lo].transpose(1, 2, 0)
    out = full.reshape(B, T, C, N).transpose(0, 3, 1, 2)
    return np.ascontiguousarray(out)
